# revision 20
# baseline (speedup 1.0000x reference)
"""8-bit ripple-carry adder on {0,1} floats — Trainium2 Bass kernel.

Problem: A, B [N=2^23, 8] f32 bits (MSB first), Cin [N,1] f32.
reference ripples from bit 7 (LSB) to bit 0 (MSB):
    t = a + b + c ; s = t mod 2 ; c' = t >= 2
Returns (sums [N,8], carry [N,1]) like the reference.

Sharding: batch dim N split evenly across 8 NeuronCores, no communication.

Key measured facts driving the design (trn2, DVE @0.96GHz):
  - compact bf16 tensor_tensor = 0.52 cyc/elem (2x mode); f32 / strided = 1
  - single-element strided WRITES cost ~2 cyc/elem (reads are free)
  - the ripple chain is serial: each dependent DVE op adds a ~0.3-0.4us
    semaphore stall, so two independent chains are interleaved (chunk pairs)
    to fill the gaps
  - HWDGE/SWDGE queues issue FIFO per engine: a store waiting on compute
    must never sit in front of a load. Queue map: gpsimd = AB cast-loads
    (f32->bf16; HBM side still reads full f32), sync = Cin loads,
    scalar = ACT ops + both stores, vector = compute.

Host packs A|B row-wise into one [NS,16] tensor per core so each chunk is a
single load DMA. Chunks are 128*R rows as SBUF tiles [128,16R] bf16
(partition p holds R rows; bit i of A = t[:, i::16], of B = t[:, 8+i::16]).

Per chunk: U = A + B for all bits in ONE 2x-mode op, then per bit
    t2 = U_i + carry          DVE (strided read, compact bf16 write)
    carry = t2 >= 2           DVE tensor_scalar (bf16 4x mode)
    s_i:  bits 7..3           ACT sin(pi/2*t2 - pi) then |.| (the -pi bias
                              avoids the bad spline region at 3*pi/2)
          bits 2..0           DVE STT  s = (carry * -2) + t2
Tiny "primer" ops (memset / ACT copy into disjoint columns) absorb
WAR-vs-store waits so real ops keep one semaphore wait each (the HW wait
slot; extras become EventSemaphore instructions via bacc).
"""

import math
import os

import numpy as np

N_TOTAL = 8388608
N_CORES = 8
NS = N_TOTAL // N_CORES  # rows per core

R = 512  # rows per partition per chunk
ACT_BITS = (7, 6, 5, 4, 3)  # sum-extraction on ACT; rest on DVE

_CACHE = {}


def _build(R: int):
    import concourse.tile as tile
    from concourse import bacc, mybir

    f32 = mybir.dt.float32
    bf16 = mybir.dt.bfloat16
    chunk_rows = 128 * R
    n_chunks = NS // chunk_rows
    assert NS % chunk_rows == 0 and n_chunks % 2 == 0

    nc = bacc.Bacc(None)
    AB = nc.declare_dram_parameter("AB", [NS, 16], f32, isOutput=False)
    Cin = nc.declare_dram_parameter("Cin", [NS, 1], f32, isOutput=False)
    S = nc.declare_dram_parameter("sums", [NS, 8], f32, isOutput=True)
    CO = nc.declare_dram_parameter("carry", [NS, 1], f32, isOutput=True)

    def chunk_views(r0, Rc):
        rows = 128 * Rc
        abv = AB[r0 : r0 + rows, :].rearrange("(p r) m -> p (r m)", p=128)
        cv = Cin[r0 : r0 + rows, :].rearrange("(p r) m -> p (r m)", p=128)
        sv = S[r0 : r0 + rows, :].rearrange("(p r) m -> p (r m)", p=128)
        cov = CO[r0 : r0 + rows, :].rearrange("(p r) m -> p (r m)", p=128)
        return abv, cv, sv, cov

    HALF_PI = math.pi / 2.0
    Sin = mybir.ActivationFunctionType.Sin
    Abs = mybir.ActivationFunctionType.Abs
    is_ge = mybir.AluOpType.is_ge
    mult = mybir.AluOpType.mult
    add = mybir.AluOpType.add

    with tile.TileContext(nc) as tc:
        with (
            tc.tile_pool(name="const", bufs=1) as const_pool,
            tc.tile_pool(name="ab", bufs=2) as ab_pool,
            tc.tile_pool(name="io", bufs=3) as io_pool,
            tc.tile_pool(name="tmp", bufs=3) as tmp_pool,
            tc.tile_pool(name="cnp", bufs=6) as cn_pool,
        ):
            z16 = const_pool.tile([128, 16], f32, tag="z16")
            nc.vector.memset(z16[:], 0.0)
            npi = const_pool.tile([128, 1], f32, tag="npi")
            nc.vector.memset(npi[:], -math.pi)

            class Chunk:
                pass

            def start_chunk(spec):
                r0, Rc = spec
                ch = Chunk()
                ch.R = Rc
                ch.views = chunk_views(r0, Rc)
                abv, cv, sv, cov = ch.views
                ch.tAB = ab_pool.tile([128, 16 * Rc], f32, tag="AB")
                nc.sync.dma_start(out=ch.tAB[:], in_=abv)
                ch.tC = io_pool.tile([128, Rc], f32, tag="Cin")
                nc.sync.dma_start(out=ch.tC[:], in_=cv)
                ch.tOUT = io_pool.tile([128, 8 * Rc], f32, tag="OUT")
                # disjoint store-WAR absorbers (ACT cols 3..7, DVE cols 0..2)
                nc.scalar.copy(ch.tOUT[:][:, 3:8], z16[:][:, 0:5])
                nc.vector.memset(ch.tOUT[:][:, 0:3], 0.0)
                ch.t2a = tmp_pool.tile([128, 8 * Rc], bf16, tag="t2")
                nc.vector.memset(ch.t2a[:][:, 0::Rc], 0.0)
                ch.cnf = tmp_pool.tile([128, Rc], f32, tag="cnf")
                nc.vector.memset(ch.cnf[:][:, 0:1], 0.0)
                ch.carry = tmp_pool.tile([128, Rc], bf16, tag="c0")
                nc.vector.tensor_copy(ch.carry[:], ch.tC[:])
                # U = A + B for all 8 bit positions in one 2x-mode op
                ch.U = tmp_pool.tile([128, 8 * Rc], bf16, tag="U")
                abv = ch.tAB[:].rearrange("p (r m) -> p r m", m=16)
                uv = ch.U[:].rearrange("p (r m) -> p r m", m=8)
                nc.vector.tensor_tensor(
                    uv[:, :, 0:8], abv[:, :, 0:8], abv[:, :, 8:16], add
                )
                return ch

            def chain_step(ch, k, i):
                Rc = ch.R
                t2 = ch.t2a[:][:, k * Rc : (k + 1) * Rc]
                nc.vector.tensor_add(t2, ch.U[:][:, i::8], ch.carry[:])
                if i > 0:
                    cn = cn_pool.tile([128, Rc], bf16, tag="cn")
                else:
                    cn = ch.cnf
                nc.vector.tensor_scalar(cn[:], t2, 2.0, None, is_ge)
                if i in ACT_BITS:
                    sr = cn_pool.tile([128, Rc], bf16, tag="sr")
                    nc.scalar.activation(
                        sr[:], t2, Sin, scale=HALF_PI, bias=npi[:]
                    )
                    nc.scalar.activation(ch.tOUT[:][:, i::8], sr[:], Abs)
                else:
                    nc.vector.scalar_tensor_tensor(
                        ch.tOUT[:][:, i::8], cn[:], -2.0, t2, mult, add
                    )
                ch.carry = cn

            def finish_chunk(ch):
                abv, cv, sv, cov = ch.views
                nc.scalar.dma_start(out=sv, in_=ch.tOUT[:])
                nc.scalar.dma_start(out=cov, in_=ch.cnf[:])

            # Rolling 2-deep software pipeline with half-chain stagger: one
            # chunk rides bits 7..4 while the other rides 3..0, so chain
            # semaphore stalls, ACT work, loads and stores all spread out
            # instead of bursting at pair boundaries.
            bits = [7, 6, 5, 4, 3, 2, 1, 0]
            # head/tail taper: small chunks at the start fill the pipeline
            # early; small chunks at the end shrink the ramp-down.
            specs = []
            row = 0
            for Rc in [256, 256] + [R] * (n_chunks - 2) + [256, 128, 128]:
                specs.append((row, Rc))
                row += 128 * Rc
            assert row == NS
            active = []
            next_c = 0

            def advance(ch):
                k = ch.step
                chain_step(ch, k, bits[k])
                ch.step += 1
                return ch.step == 8

            ch = start_chunk(specs[next_c])
            ch.step = 0
            active.append(ch)
            next_c += 1
            for _ in range(4):
                advance(active[0])
            ch = start_chunk(specs[next_c])
            ch.step = 0
            active.append(ch)
            next_c += 1

            while active:
                for ch in list(active):
                    if advance(ch):
                        finish_chunk(ch)
                        active.remove(ch)
                        if next_c < len(specs):
                            nch = start_chunk(specs[next_c])
                            nch.step = 0
                            active.append(nch)
                            next_c += 1
    nc.finalize()
    return nc


def _get_nc():
    key = ("v7", R)
    if key not in _CACHE:
        _CACHE[key] = _build(R)
    return _CACHE[key]


def kernel(A, B, Cin, _trace=False):
    from concourse.bass_utils import run_bass_kernel_spmd

    A = np.asarray(A, dtype=np.float32)
    B = np.asarray(B, dtype=np.float32)
    Cin = np.ascontiguousarray(np.asarray(Cin, dtype=np.float32))
    assert A.shape == (N_TOTAL, 8) and B.shape == (N_TOTAL, 8)
    assert Cin.shape == (N_TOTAL, 1)

    AB = np.empty((N_TOTAL, 16), dtype=np.float32)
    AB[:, :8] = A
    AB[:, 8:] = B

    nc = _get_nc()

    in_maps = []
    for i in range(N_CORES):
        lo, hi = i * NS, (i + 1) * NS
        in_maps.append({"AB": AB[lo:hi], "Cin": Cin[lo:hi]})

    res = run_bass_kernel_spmd(
        nc, in_maps, core_ids=list(range(N_CORES)), trace=_trace
    )

    sums = np.empty((N_TOTAL, 8), dtype=np.float32)
    carry = np.empty((N_TOTAL, 1), dtype=np.float32)
    for i in range(N_CORES):
        lo, hi = i * NS, (i + 1) * NS
        sums[lo:hi] = res.results[i]["sums"]
        carry[lo:hi] = res.results[i]["carry"]

    if _trace:
        kernel.last_exec_time_ns = res.exec_time_ns
    return sums, carry


kernel.last_exec_time_ns = None


# revision 21
# speedup vs baseline: 1.0128x; 1.0128x over previous
"""8-bit ripple-carry adder on {0,1} floats — Trainium2 Bass kernel.

Problem: A, B [N=2^23, 8] f32 bits (MSB first), Cin [N,1] f32.
reference ripples from bit 7 (LSB) to bit 0 (MSB):
    t = a + b + c ; s = t mod 2 ; c' = t >= 2
Returns (sums [N,8], carry [N,1]) like the reference.

Sharding: batch dim N split evenly across 8 NeuronCores, no communication.

Key measured facts driving the design (trn2, DVE @0.96GHz):
  - compact bf16 tensor_tensor = 0.52 cyc/elem (2x mode); f32 / strided = 1
  - single-element strided WRITES cost ~2 cyc/elem (reads are free)
  - the ripple chain is serial: each dependent DVE op adds a ~0.3-0.4us
    semaphore stall, so two independent chains are interleaved (chunk pairs)
    to fill the gaps
  - HWDGE/SWDGE queues issue FIFO per engine: a store waiting on compute
    must never sit in front of a load. Queue map: gpsimd = AB cast-loads
    (f32->bf16; HBM side still reads full f32), sync = Cin loads,
    scalar = ACT ops + both stores, vector = compute.

Host packs A|B row-wise into one [NS,16] tensor per core so each chunk is a
single load DMA. Chunks are 128*R rows as SBUF tiles [128,16R] bf16
(partition p holds R rows; bit i of A = t[:, i::16], of B = t[:, 8+i::16]).

Per chunk: U = A + B for all bits in ONE 2x-mode op, then per bit
    t2 = U_i + carry          DVE (strided read, compact bf16 write)
    carry = t2 >= 2           DVE tensor_scalar (bf16 4x mode)
    s_i:  bits 7..3           ACT sin(pi/2*t2 - pi) then |.| (the -pi bias
                              avoids the bad spline region at 3*pi/2)
          bits 2..0           DVE STT  s = (carry * -2) + t2
Tiny "primer" ops (memset / ACT copy into disjoint columns) absorb
WAR-vs-store waits so real ops keep one semaphore wait each (the HW wait
slot; extras become EventSemaphore instructions via bacc).
"""

import math
import os

import numpy as np

N_TOTAL = 8388608
N_CORES = 8
NS = N_TOTAL // N_CORES  # rows per core

R = 512  # rows per partition per chunk
ACT_BITS = (7, 6, 5, 4, 3)  # sum-extraction on ACT; rest on DVE

_CACHE = {}


def _build(R: int):
    import concourse.tile as tile
    from concourse import bacc, mybir

    f32 = mybir.dt.float32
    bf16 = mybir.dt.bfloat16
    chunk_rows = 128 * R
    n_chunks = NS // chunk_rows
    assert NS % chunk_rows == 0 and n_chunks % 2 == 0

    nc = bacc.Bacc(None)
    AB = nc.declare_dram_parameter("AB", [NS, 17], f32, isOutput=False)
    S = nc.declare_dram_parameter("sums", [NS, 8], f32, isOutput=True)
    CO = nc.declare_dram_parameter("carry", [NS, 1], f32, isOutput=True)

    def chunk_views(r0, Rc):
        rows = 128 * Rc
        abv = AB[r0 : r0 + rows, :].rearrange("(p r) m -> p (r m)", p=128)
        sv = S[r0 : r0 + rows, :].rearrange("(p r) m -> p (r m)", p=128)
        cov = CO[r0 : r0 + rows, :].rearrange("(p r) m -> p (r m)", p=128)
        return abv, sv, cov

    HALF_PI = math.pi / 2.0
    Sin = mybir.ActivationFunctionType.Sin
    Abs = mybir.ActivationFunctionType.Abs
    is_ge = mybir.AluOpType.is_ge
    mult = mybir.AluOpType.mult
    add = mybir.AluOpType.add

    with tile.TileContext(nc) as tc:
        with (
            tc.tile_pool(name="const", bufs=1) as const_pool,
            tc.tile_pool(name="ab", bufs=2) as ab_pool,
            tc.tile_pool(name="io", bufs=3) as io_pool,
            tc.tile_pool(name="tmp", bufs=3) as tmp_pool,
            tc.tile_pool(name="cnp", bufs=6) as cn_pool,
        ):
            z16 = const_pool.tile([128, 16], f32, tag="z16")
            nc.vector.memset(z16[:], 0.0)
            npi = const_pool.tile([128, 1], f32, tag="npi")
            nc.vector.memset(npi[:], -math.pi)

            class Chunk:
                pass

            def start_chunk(spec):
                r0, Rc = spec
                ch = Chunk()
                ch.R = Rc
                ch.views = chunk_views(r0, Rc)
                abv, sv, cov = ch.views
                ch.tAB = ab_pool.tile([128, 17 * Rc], f32, tag="AB")
                nc.sync.dma_start(out=ch.tAB[:], in_=abv)
                ch.tOUT = io_pool.tile([128, 8 * Rc], f32, tag="OUT")
                # disjoint store-WAR absorbers (ACT cols 3..7, DVE cols 0..2)
                nc.scalar.copy(ch.tOUT[:][:, 3:8], z16[:][:, 0:5])
                nc.vector.memset(ch.tOUT[:][:, 0:3], 0.0)
                ch.t2a = tmp_pool.tile([128, 8 * Rc], bf16, tag="t2")
                nc.vector.memset(ch.t2a[:][:, 0::Rc], 0.0)
                ch.cnf = tmp_pool.tile([128, Rc], f32, tag="cnf")
                nc.vector.memset(ch.cnf[:][:, 0:1], 0.0)
                ch.carry = tmp_pool.tile([128, Rc], bf16, tag="c0")
                nc.vector.tensor_copy(ch.carry[:], ch.tAB[:][:, 16::17])
                # U = A + B for all 8 bit positions in one 2x-mode op
                ch.U = tmp_pool.tile([128, 8 * Rc], bf16, tag="U")
                abv = ch.tAB[:].rearrange("p (r m) -> p r m", m=17)
                uv = ch.U[:].rearrange("p (r m) -> p r m", m=8)
                nc.vector.tensor_tensor(
                    uv[:, :, 0:8], abv[:, :, 0:8], abv[:, :, 8:16], add
                )
                return ch

            def chain_step(ch, k, i):
                Rc = ch.R
                t2 = ch.t2a[:][:, k * Rc : (k + 1) * Rc]
                nc.vector.tensor_add(t2, ch.U[:][:, i::8], ch.carry[:])
                if i > 0:
                    cn = cn_pool.tile([128, Rc], bf16, tag="cn")
                else:
                    cn = ch.cnf
                nc.vector.tensor_scalar(cn[:], t2, 2.0, None, is_ge)
                if i in ACT_BITS:
                    sr = cn_pool.tile([128, Rc], bf16, tag="sr")
                    nc.scalar.activation(
                        sr[:], t2, Sin, scale=HALF_PI, bias=npi[:]
                    )
                    nc.scalar.activation(ch.tOUT[:][:, i::8], sr[:], Abs)
                else:
                    nc.vector.scalar_tensor_tensor(
                        ch.tOUT[:][:, i::8], cn[:], -2.0, t2, mult, add
                    )
                ch.carry = cn

            def finish_chunk(ch):
                abv, sv, cov = ch.views
                nc.scalar.dma_start(out=sv, in_=ch.tOUT[:])
                nc.scalar.dma_start(out=cov, in_=ch.cnf[:])

            # Rolling 2-deep software pipeline with half-chain stagger: one
            # chunk rides bits 7..4 while the other rides 3..0, so chain
            # semaphore stalls, ACT work, loads and stores all spread out
            # instead of bursting at pair boundaries.
            bits = [7, 6, 5, 4, 3, 2, 1, 0]
            # head/tail taper: small chunks at the start fill the pipeline
            # early; small chunks at the end shrink the ramp-down.
            specs = []
            row = 0
            for Rc in [R] * n_chunks:
                specs.append((row, Rc))
                row += 128 * Rc
            assert row == NS
            active = []
            next_c = 0

            def advance(ch):
                k = ch.step
                chain_step(ch, k, bits[k])
                ch.step += 1
                return ch.step == 8

            ch = start_chunk(specs[next_c])
            ch.step = 0
            active.append(ch)
            next_c += 1
            for _ in range(4):
                advance(active[0])
            ch = start_chunk(specs[next_c])
            ch.step = 0
            active.append(ch)
            next_c += 1

            while active:
                for ch in list(active):
                    if advance(ch):
                        finish_chunk(ch)
                        active.remove(ch)
                        if next_c < len(specs):
                            nch = start_chunk(specs[next_c])
                            nch.step = 0
                            active.append(nch)
                            next_c += 1
    nc.finalize()
    return nc


def _get_nc():
    key = ("v8", R)
    if key not in _CACHE:
        _CACHE[key] = _build(R)
    return _CACHE[key]


def kernel(A, B, Cin, _trace=False):
    from concourse.bass_utils import run_bass_kernel_spmd

    A = np.asarray(A, dtype=np.float32)
    B = np.asarray(B, dtype=np.float32)
    Cin = np.ascontiguousarray(np.asarray(Cin, dtype=np.float32))
    assert A.shape == (N_TOTAL, 8) and B.shape == (N_TOTAL, 8)
    assert Cin.shape == (N_TOTAL, 1)

    AB = np.empty((N_TOTAL, 17), dtype=np.float32)
    AB[:, :8] = A
    AB[:, 8:16] = B
    AB[:, 16:] = Cin

    nc = _get_nc()

    in_maps = []
    for i in range(N_CORES):
        lo, hi = i * NS, (i + 1) * NS
        in_maps.append({"AB": AB[lo:hi]})

    res = run_bass_kernel_spmd(
        nc, in_maps, core_ids=list(range(N_CORES)), trace=_trace
    )

    sums = np.empty((N_TOTAL, 8), dtype=np.float32)
    carry = np.empty((N_TOTAL, 1), dtype=np.float32)
    for i in range(N_CORES):
        lo, hi = i * NS, (i + 1) * NS
        sums[lo:hi] = res.results[i]["sums"]
        carry[lo:hi] = res.results[i]["carry"]

    if _trace:
        kernel.last_exec_time_ns = res.exec_time_ns
    return sums, carry


kernel.last_exec_time_ns = None


# revision 22
# speedup vs baseline: 1.1182x; 1.1040x over previous
"""8-bit ripple-carry adder on {0,1} floats — Trainium2 Bass kernel.

Problem: A, B [N=2^23, 8] f32 bits (MSB first), Cin [N,1] f32.
reference ripples from bit 7 (LSB) to bit 0 (MSB):
    t = a + b + c ; s = t mod 2 ; c' = t >= 2
Returns (sums [N,8], carry [N,1]) like the reference.

Sharding: batch dim N split evenly across 8 NeuronCores, no communication.

Key measured facts driving the design (trn2, DVE @0.96GHz):
  - compact bf16 tensor_tensor = 0.52 cyc/elem (2x mode); f32 / strided = 1
  - single-element strided WRITES cost ~2 cyc/elem (reads are free)
  - the ripple chain is serial: each dependent DVE op adds a ~0.3-0.4us
    semaphore stall, so two independent chains are interleaved (chunk pairs)
    to fill the gaps
  - HWDGE/SWDGE queues issue FIFO per engine: a store waiting on compute
    must never sit in front of a load. Queue map: gpsimd = AB cast-loads
    (f32->bf16; HBM side still reads full f32), sync = Cin loads,
    scalar = ACT ops + both stores, vector = compute.

Host packs A|B row-wise into one [NS,16] tensor per core so each chunk is a
single load DMA. Chunks are 128*R rows as SBUF tiles [128,16R] bf16
(partition p holds R rows; bit i of A = t[:, i::16], of B = t[:, 8+i::16]).

Per chunk: U = A + B for all bits in ONE 2x-mode op, then per bit
    t2 = U_i + carry          DVE (strided read, compact bf16 write)
    carry = t2 >= 2           DVE tensor_scalar (bf16 4x mode)
    s_i:  bits 7..3           ACT sin(pi/2*t2 - pi) then |.| (the -pi bias
                              avoids the bad spline region at 3*pi/2)
          bits 2..0           DVE STT  s = (carry * -2) + t2
Tiny "primer" ops (memset / ACT copy into disjoint columns) absorb
WAR-vs-store waits so real ops keep one semaphore wait each (the HW wait
slot; extras become EventSemaphore instructions via bacc).
"""

import math
import os

import numpy as np

N_TOTAL = 8388608
N_CORES = 8
NS = N_TOTAL // N_CORES  # rows per core

R = 512  # rows per partition per chunk
ACT_BITS = (7, 6, 5, 4, 3)  # sum-extraction on ACT; rest on DVE

_CACHE = {}


def _build(R: int):
    import concourse.tile as tile
    from concourse import bacc, mybir

    f32 = mybir.dt.float32
    bf16 = mybir.dt.bfloat16
    chunk_rows = 128 * R
    n_chunks = NS // chunk_rows
    assert NS % chunk_rows == 0 and n_chunks % 2 == 0

    nc = bacc.Bacc(None)
    AB = nc.declare_dram_parameter("AB", [NS, 16], f32, isOutput=False)
    Cin = nc.declare_dram_parameter("Cin", [NS, 1], f32, isOutput=False)
    S = nc.declare_dram_parameter("sums", [NS, 8], f32, isOutput=True)
    CO = nc.declare_dram_parameter("carry", [NS, 1], f32, isOutput=True)

    def chunk_views(r0, Rc):
        rows = 128 * Rc
        abv = AB[r0 : r0 + rows, :].rearrange("(p r) m -> p (r m)", p=128)
        cv = Cin[r0 : r0 + rows, :].rearrange("(p r) m -> p (r m)", p=128)
        sv = S[r0 : r0 + rows, :].rearrange("(p r) m -> p (r m)", p=128)
        cov = CO[r0 : r0 + rows, :].rearrange("(p r) m -> p (r m)", p=128)
        return abv, cv, sv, cov

    HALF_PI = math.pi / 2.0
    Sin = mybir.ActivationFunctionType.Sin
    Abs = mybir.ActivationFunctionType.Abs
    is_ge = mybir.AluOpType.is_ge
    mult = mybir.AluOpType.mult
    add = mybir.AluOpType.add

    with tile.TileContext(nc) as tc:
        with (
            tc.tile_pool(name="const", bufs=1) as const_pool,
            tc.tile_pool(name="ab", bufs=2) as ab_pool,
            tc.tile_pool(name="io", bufs=3) as io_pool,
            tc.tile_pool(name="tmp", bufs=3) as tmp_pool,
            tc.tile_pool(name="cnp", bufs=6) as cn_pool,
        ):
            z16 = const_pool.tile([128, 16], f32, tag="z16")
            nc.vector.memset(z16[:], 0.0)
            npi = const_pool.tile([128, 1], f32, tag="npi")
            nc.vector.memset(npi[:], -math.pi)

            class Chunk:
                pass

            def start_chunk(spec):
                r0, Rc = spec
                ch = Chunk()
                ch.R = Rc
                ch.views = chunk_views(r0, Rc)
                abv, cv, sv, cov = ch.views
                ch.tAB = ab_pool.tile([128, 16 * Rc], f32, tag="AB")
                nc.sync.dma_start(out=ch.tAB[:], in_=abv)
                ch.tC = io_pool.tile([128, Rc], f32, tag="Cin")
                nc.sync.dma_start(out=ch.tC[:], in_=cv)
                ch.tOUT = io_pool.tile([128, 8 * Rc], f32, tag="OUT")
                # disjoint store-WAR absorbers (ACT cols 3..7, DVE cols 0..2)
                nc.scalar.copy(ch.tOUT[:][:, 3:8], z16[:][:, 0:5])
                nc.vector.memset(ch.tOUT[:][:, 0:3], 0.0)
                ch.t2a = tmp_pool.tile([128, 8 * Rc], bf16, tag="t2")
                nc.vector.memset(ch.t2a[:][:, 0::Rc], 0.0)
                ch.cnf = tmp_pool.tile([128, Rc], f32, tag="cnf")
                nc.vector.memset(ch.cnf[:][:, 0:1], 0.0)
                ch.carry = tmp_pool.tile([128, Rc], bf16, tag="c0")
                nc.vector.tensor_copy(ch.carry[:], ch.tC[:])
                # U = A + B for all 8 bit positions in one 2x-mode op
                ch.U = tmp_pool.tile([128, 8 * Rc], bf16, tag="U")
                abv = ch.tAB[:].rearrange("p (r m) -> p r m", m=16)
                uv = ch.U[:].rearrange("p (r m) -> p r m", m=8)
                nc.vector.tensor_tensor(
                    uv[:, :, 0:8], abv[:, :, 0:8], abv[:, :, 8:16], add
                )
                return ch

            def chain_step(ch, k, i):
                Rc = ch.R
                t2 = ch.t2a[:][:, k * Rc : (k + 1) * Rc]
                nc.vector.tensor_add(t2, ch.U[:][:, i::8], ch.carry[:])
                if i > 0:
                    cn = cn_pool.tile([128, Rc], bf16, tag="cn")
                else:
                    cn = ch.cnf
                nc.vector.tensor_scalar(cn[:], t2, 2.0, None, is_ge)
                if i in ACT_BITS:
                    sr = cn_pool.tile([128, Rc], bf16, tag="sr")
                    nc.scalar.activation(
                        sr[:], t2, Sin, scale=HALF_PI, bias=npi[:]
                    )
                    nc.scalar.activation(ch.tOUT[:][:, i::8], sr[:], Abs)
                else:
                    nc.vector.scalar_tensor_tensor(
                        ch.tOUT[:][:, i::8], cn[:], -2.0, t2, mult, add
                    )
                ch.carry = cn

            def finish_chunk(ch):
                abv, cv, sv, cov = ch.views
                nc.scalar.dma_start(out=sv, in_=ch.tOUT[:])
                nc.scalar.dma_start(out=cov, in_=ch.cnf[:])

            # Rolling 2-deep software pipeline with half-chain stagger: one
            # chunk rides bits 7..4 while the other rides 3..0, so chain
            # semaphore stalls, ACT work, loads and stores all spread out
            # instead of bursting at pair boundaries.
            bits = [7, 6, 5, 4, 3, 2, 1, 0]
            # head/tail taper: small chunks at the start fill the pipeline
            # early; small chunks at the end shrink the ramp-down.
            specs = []
            row = 0
            for Rc in [R] * n_chunks:
                specs.append((row, Rc))
                row += 128 * Rc
            assert row == NS
            active = []
            next_c = 0

            def advance(ch):
                k = ch.step
                chain_step(ch, k, bits[k])
                ch.step += 1
                return ch.step == 8

            ch = start_chunk(specs[next_c])
            ch.step = 0
            active.append(ch)
            next_c += 1
            for _ in range(4):
                advance(active[0])
            ch = start_chunk(specs[next_c])
            ch.step = 0
            active.append(ch)
            next_c += 1

            while active:
                for ch in list(active):
                    if advance(ch):
                        finish_chunk(ch)
                        active.remove(ch)
                        if next_c < len(specs):
                            nch = start_chunk(specs[next_c])
                            nch.step = 0
                            active.append(nch)
                            next_c += 1
    nc.finalize()
    return nc


def _get_nc():
    key = ("v9", R)
    if key not in _CACHE:
        _CACHE[key] = _build(R)
    return _CACHE[key]


def kernel(A, B, Cin, _trace=False):
    from concourse.bass_utils import run_bass_kernel_spmd

    A = np.asarray(A, dtype=np.float32)
    B = np.asarray(B, dtype=np.float32)
    Cin = np.ascontiguousarray(np.asarray(Cin, dtype=np.float32))
    assert A.shape == (N_TOTAL, 8) and B.shape == (N_TOTAL, 8)
    assert Cin.shape == (N_TOTAL, 1)

    AB = np.empty((N_TOTAL, 16), dtype=np.float32)
    AB[:, :8] = A
    AB[:, 8:] = B

    nc = _get_nc()

    in_maps = []
    for i in range(N_CORES):
        lo, hi = i * NS, (i + 1) * NS
        in_maps.append({"AB": AB[lo:hi], "Cin": Cin[lo:hi]})

    res = run_bass_kernel_spmd(
        nc, in_maps, core_ids=list(range(N_CORES)), trace=_trace
    )

    sums = np.empty((N_TOTAL, 8), dtype=np.float32)
    carry = np.empty((N_TOTAL, 1), dtype=np.float32)
    for i in range(N_CORES):
        lo, hi = i * NS, (i + 1) * NS
        sums[lo:hi] = res.results[i]["sums"]
        carry[lo:hi] = res.results[i]["carry"]

    if _trace:
        kernel.last_exec_time_ns = res.exec_time_ns
    return sums, carry


kernel.last_exec_time_ns = None
